# revision 2
# baseline (speedup 1.0000x reference)
"""Causal RoPE attention (B=4, T=2048, D=2048, H=16, Dh=128) on 8 trn2 cores.

Sharding (tensor-parallel over heads, no collectives): core c handles batch
b = c//2 and head-half g = c%2 (heads g*8..g*8+7).  Each core projects
Q/K/V for its 8 heads over the full sequence, applies RoPE, runs causal
flash-style attention, and multiplies by its half of w_o (row-split),
producing a PARTIAL [T, D] output (bf16).  The host upcasts and adds the
two partials per batch.  All matmuls run in bf16 (fp32 PSUM
accumulation).

v6 schedule (one core):
- prefetch wk0/wq0, xT column chunk 0, wv, rope tables, rest of xT.
- head-0 K/Q projection chunks interleaved with V-projection brackets so
  PE work matches DMA arrival order.
- per head: K/Q proj + RoPE -> causal attention (512-query chunks in
  order [1,0,2,3]; 128-key blocks; exp on ACT, 0/1 diagonal masks on
  gpsimd, AV + ones-denominator accumulated in PSUM).  Normalized attn
  chunks stream to a DRAM buffer attnD laid out [rb, p, h*128+c] so the
  output projection can re-read them contiguously.
- output projection in column pairs: per rb, one contiguous attn tile
  feeds both 512-column halves (halves attn re-read traffic).
- PSUM tags: pk(2) st(3) au(1) dn(2) = 8 banks; V proj / out proj reuse pk.
"""

import sys

sys.path.insert(0, "/opt/trn_rl_repo")

import numpy as np

D = 2048
T = 2048
H = 16
HL = 8             # heads per core (local)
DH = 128
B = 4
NKB = T // 128     # 16 key blocks
NQC = T // 512     # 4 query chunks
SCALE = 1.0 / np.sqrt(128.0)
QC_ORDER = (1, 0, 2, 3)

_compiled = {}


def _build_nc():
    import concourse.bacc as bacc
    from concourse import mybir
    from concourse.tile import TileContext

    BF16 = mybir.dt.bfloat16
    F32 = mybir.dt.float32
    EXP = mybir.ActivationFunctionType.Exp

    nc = bacc.Bacc(trn_type="TRN2")

    xT_d = nc.dram_tensor("xT", [D, T], BF16, kind="ExternalInput")
    wqbh_d = nc.dram_tensor("wqbh", [HL, 128, 16, 128], BF16, kind="ExternalInput")
    wkbh_d = nc.dram_tensor("wkbh", [HL, 128, 16, 128], BF16, kind="ExternalInput")
    wvT_d = nc.dram_tensor("wvT", [D, HL * DH], BF16, kind="ExternalInput")
    woT_d = nc.dram_tensor("woT", [HL * DH, D], BF16, kind="ExternalInput")
    cosk_d = nc.dram_tensor("cosk", [DH, T], BF16, kind="ExternalInput")
    sink_d = nc.dram_tensor("sink", [DH, T], BF16, kind="ExternalInput")
    mask_d = nc.dram_tensor("mask", [128, 128], BF16, kind="ExternalInput")
    ones_d = nc.dram_tensor("ones", [128, 1], BF16, kind="ExternalInput")
    out_d = nc.dram_tensor("out", [T, D], BF16, kind="ExternalOutput")

    with TileContext(nc) as tc:
        with tc.tile_pool(name="persist", bufs=1) as persist, \
             tc.tile_pool(name="dram", bufs=1, space="DRAM") as dram, \
             tc.tile_pool(name="wqk", bufs=2) as wqkp, \
             tc.tile_pool(name="wop", bufs=2) as wop, \
             tc.tile_pool(name="wvp", bufs=1) as wvp, \
             tc.tile_pool(name="ktq", bufs=2) as ktqp, \
             tc.tile_pool(name="rt", bufs=2) as rtp, \
             tc.tile_pool(name="pp", bufs=3) as ppool, \
             tc.tile_pool(name="nt", bufs=2) as ntp, \
             tc.tile_pool(name="pj", bufs=2, space="PSUM") as pjp, \
             tc.tile_pool(name="ps", bufs=3, space="PSUM") as psp, \
             tc.tile_pool(name="pa", bufs=1, space="PSUM") as pap, \
             tc.tile_pool(name="pd", bufs=2, space="PSUM") as pdp:
            # ---- persistent SBUF residents -------------------------------
            xT_sb = [
                persist.tile([128, T], BF16, name=f"xT{d}", tag=f"xT{d}")
                for d in range(16)
            ]
            v_sb = [
                persist.tile([128, HL * DH], BF16, name=f"v{tb}", tag=f"v{tb}")
                for tb in range(NKB)
            ]
            ck = persist.tile([128, T], BF16, name="ck", tag="ck")
            sk = persist.tile([128, T], BF16, name="sk", tag="sk")
            msk = persist.tile([128, 128], BF16, name="msk", tag="msk")
            ones_sb = persist.tile([128, 1], BF16, name="ones", tag="ones")
            # attn DRAM buffer: [rb, p(dh), h*128+c(t within rb)]
            attnD = dram.tile([NKB, 128, HL * 128], BF16, name="attnD")

            def wqk_tiles(h):
                wk = wqkp.tile([128, 16, 128], BF16, tag="wk", name=f"wk{h}")
                wq = wqkp.tile([128, 16, 128], BF16, tag="wq", name=f"wq{h}")
                nc.sync.dma_start(wk[:], wkbh_d[h])
                nc.sync.dma_start(wq[:], wqbh_d[h])
                return wk, wq

            # prefetch order: head-0 weights, first xT chunk, wv, rope
            # tables, rest of xT, masks
            wk0, wq0 = wqk_tiles(0)
            for d in range(16):
                nc.sync.dma_start(xT_sb[d][:, 0:512],
                                  xT_d[d * 128:(d + 1) * 128, 0:512])
            wv_sb = {}
            for d in range(16):
                wt = wvp.tile([128, 512], BF16, tag=f"wv{d}", name=f"wv{d}_0")
                nc.sync.dma_start(wt[:], wvT_d[d * 128:(d + 1) * 128, 0:512])
                wv_sb[(0, d)] = wt
            nc.sync.dma_start(ck[:], cosk_d[:])
            nc.sync.dma_start(sk[:], sink_d[:])
            for tcn in range(1, 4):
                tsl = slice(tcn * 512, (tcn + 1) * 512)
                for d in range(16):
                    nc.sync.dma_start(xT_sb[d][:, tsl],
                                      xT_d[d * 128:(d + 1) * 128, tsl])
            nc.sync.dma_start(msk[:], mask_d[:])
            nc.sync.dma_start(ones_sb[:], ones_d[:])

            def emit_proj_chunk(wt, dst, tcn):
                tsl = slice(tcn * 512, (tcn + 1) * 512)
                pk = pjp.tile([128, 512], F32, tag="pk")
                for d in range(16):
                    nc.tensor.matmul(pk[:], wt[:, d, :], xT_sb[d][:, tsl],
                                     start=(d == 0), stop=(d == 15))
                t1 = rtp.tile([128, 512], F32, tag="t1")
                t2 = rtp.tile([128, 512], F32, tag="t2")
                nc.vector.tensor_mul(t1[:], pk[:], ck[:, tsl])
                nc.vector.tensor_mul(t2[0:64, :], pk[64:128, :], sk[0:64, tsl])
                nc.vector.tensor_mul(t2[64:128, :], pk[0:64, :], sk[64:128, tsl])
                nc.gpsimd.tensor_add(dst[:, tsl], t1[:], t2[:])

            def emit_v_bracket(oc, tb0):
                osl = slice(oc * 512, (oc + 1) * 512)
                for tb in range(tb0, tb0 + 4):
                    pv = pjp.tile([128, 512], F32, tag="pk")
                    for d in range(16):
                        nc.tensor.matmul(
                            pv[:], xT_sb[d][:, tb * 128:(tb + 1) * 128],
                            wv_sb[(oc, d)][:], start=(d == 0), stop=(d == 15))
                    nc.scalar.copy(v_sb[tb][:, osl], pv[:])

            def emit_attn(h, kt, qt):
                for qc in QC_ORDER:
                    qsl = slice(qc * 512, (qc + 1) * 512)
                    nkb = 4 * (qc + 1)
                    au = pap.tile([128, 512], F32, tag="au", name=f"au{h}_{qc}")
                    dn = pdp.tile([1, 512], F32, tag="dn", name=f"dn{h}_{qc}")
                    for kb in range(nkb):
                        j = kb - (nkb - 4)
                        qoff = max(0, j) * 128
                        w = 512 - qoff
                        st = psp.tile([128, 512], F32, tag="st")
                        nc.tensor.matmul(
                            st[:, 0:w], kt[:, kb * 128:(kb + 1) * 128],
                            qt[:, qc * 512 + qoff:(qc + 1) * 512],
                            start=True, stop=True)
                        pe = ppool.tile([128, 512], BF16, tag="pe")
                        nc.scalar.activation(pe[:, 0:w], st[:, 0:w], EXP,
                                             scale=SCALE)
                        if j >= 0:
                            # triangular sub-block: mask first 128 columns
                            nc.gpsimd.tensor_mul(pe[:, 0:128], pe[:, 0:128],
                                                 msk[:])
                        nc.tensor.matmul(
                            au[:, qoff:512], v_sb[kb][:, h * 128:(h + 1) * 128],
                            pe[:, 0:w], start=(kb == 0), stop=(kb == nkb - 1))
                        nc.tensor.matmul(
                            dn[:, qoff:512], ones_sb[:], pe[:, 0:w],
                            start=(kb == 0), stop=(kb == nkb - 1))
                    # free the au bank fast (plain copy, no recip dependency),
                    # then normalize off-PSUM and stream to DRAM
                    aus = ntp.tile([128, 512], BF16, tag="aus")
                    nc.vector.tensor_scalar_mul(aus[:], au[:], 1.0)
                    rec = ntp.tile([1, 512], F32, tag="rec")
                    nc.vector.reciprocal(rec[:], dn[:])
                    rbc = ntp.tile([128, 512], F32, tag="rbc")
                    nc.gpsimd.partition_broadcast(rbc[:], rec[:])
                    ao = ntp.tile([128, 512], BF16, tag="ao")
                    nc.vector.tensor_mul(ao[:], aus[:], rbc[:])
                    for rbl in range(4):
                        rb = qc * 4 + rbl
                        nc.sync.dma_start(
                            attnD[rb, :, h * 128:(h + 1) * 128],
                            ao[:, rbl * 128:(rbl + 1) * 128])

            # ---- head 0 projection interleaved with V projection ---------
            kt0 = ktqp.tile([128, T], BF16, tag="kt", name="kt0")
            qt0 = ktqp.tile([128, T], BF16, tag="qt", name="qt0")
            for tcn in range(4):
                emit_proj_chunk(wk0, kt0, tcn)
                emit_v_bracket(0, 4 * tcn)
            for d in range(16):
                wt = wvp.tile([128, 512], BF16, tag=f"wv{d}", name=f"wv{d}_1")
                nc.sync.dma_start(wt[:], wvT_d[d * 128:(d + 1) * 128, 512:1024])
                wv_sb[(1, d)] = wt
            for tcn in range(4):
                emit_proj_chunk(wq0, qt0, tcn)
                emit_v_bracket(1, 4 * tcn)

            wk, wq = wqk_tiles(1)
            emit_attn(0, kt0, qt0)
            for h in range(1, HL):
                ktn = ktqp.tile([128, T], BF16, tag="kt", name=f"kt{h}")
                qtn = ktqp.tile([128, T], BF16, tag="qt", name=f"qt{h}")
                for tcn in range(4):
                    emit_proj_chunk(wk, ktn, tcn)
                for tcn in range(4):
                    emit_proj_chunk(wq, qtn, tcn)
                if h + 1 < HL:
                    wk, wq = wqk_tiles(h + 1)
                else:
                    # prefetch first output-projection weight pair instead
                    wo_sb = {}
                    for hh in range(HL):
                        for oc in range(2):
                            wt = wop.tile([128, 512], BF16, tag=f"wo{hh}",
                                          name=f"wo{hh}_{oc}")
                            nc.sync.dma_start(
                                wt[:], woT_d[hh * 128:(hh + 1) * 128,
                                             oc * 512:(oc + 1) * 512])
                            wo_sb[(oc, hh)] = wt
                emit_attn(h, ktn, qtn)

            # ---- output projection (column pairs) ------------------------
            # out[r, o] = sum_h sum_dh attn[h][dh, r] * woT[h*128+dh, o]
            with tc.tile_pool(name="oev", bufs=3) as oevp, \
                 tc.tile_pool(name="arb", bufs=3) as arbp:
                for pair in range(2):
                    if pair == 1:
                        wo_sb = {}
                        for hh in range(HL):
                            for pc in range(2):
                                oc = 2 + pc
                                wt = wop.tile([128, 512], BF16, tag=f"wo{hh}",
                                              name=f"wo{hh}_{oc}")
                                nc.sync.dma_start(
                                    wt[:], woT_d[hh * 128:(hh + 1) * 128,
                                                 oc * 512:(oc + 1) * 512])
                                wo_sb[(pc, hh)] = wt
                    for rb in range(NKB):
                        at = arbp.tile([128, HL * 128], BF16, tag="at",
                                       name=f"at{pair}_{rb}")
                        nc.sync.dma_start(at[:], attnD[rb])
                        for pc in range(2):
                            oc = 2 * pair + pc
                            osl = slice(oc * 512, (oc + 1) * 512)
                            po = pjp.tile([128, 512], F32, tag="pk")
                            for hh in range(HL):
                                nc.tensor.matmul(
                                    po[:], at[:, hh * 128:(hh + 1) * 128],
                                    wo_sb[(pc, hh)][:],
                                    start=(hh == 0), stop=(hh == HL - 1))
                            oo = oevp.tile([128, 512], BF16, tag="oo")
                            nc.scalar.copy(oo[:], po[:])
                            nc.sync.dma_start(
                                out_d[rb * 128:(rb + 1) * 128, osl], oo[:])

    nc.compile()
    return nc


def _host_prep(x, rope_cos, rope_sin, w_q, w_k, w_v, w_o):
    import ml_dtypes

    bf16 = ml_dtypes.bfloat16
    x = np.asarray(x, dtype=np.float32)
    cosT = np.ascontiguousarray(rope_cos.T.astype(bf16))   # [128, T]
    sinT = np.asarray(rope_sin.T, dtype=np.float32).copy()
    sinT[:64] = -sinT[:64]
    sinT = np.ascontiguousarray(sinT.astype(bf16))

    # masks for the 4 diagonal key blocks of each 512-query chunk:
    # allowed (1.0) iff q_local >= kbd*128 + k_local
    q = np.arange(128)[None, :]
    k = np.arange(128)[:, None]
    m01 = (q >= k).astype(bf16)

    ones = np.ones((128, 1), dtype=bf16)

    halves = {}
    for g in range(2):
        hsl = slice(g * 8 * DH, (g + 1) * 8 * DH)

        def byhead(w):
            # w [out, in]; pre-swizzled lhsT layout [hl, p, dblk, c] so the
            # on-device weight DMA is a contiguous copy
            wt = w.T[:, hsl]                       # [D, 1024]
            return np.ascontiguousarray(
                wt.reshape(16, 128, HL, DH).transpose(2, 1, 0, 3).astype(bf16))

        halves[g] = {
            "wqbh": byhead(w_q),
            "wkbh": byhead(w_k),
            "wvT": np.ascontiguousarray(w_v.T[:, hsl].astype(bf16)),
            "woT": np.ascontiguousarray(w_o.T[hsl, :].astype(bf16)),
        }

    xT = {}
    for b in range(B):
        xT[b] = np.ascontiguousarray(x[b].T.astype(bf16))  # [D, T]

    in_maps = []
    for c in range(8):
        b, g = c // 2, c % 2
        m = dict(halves[g])
        m.update({
            "xT": xT[b],
            "cosk": cosT,
            "sink": sinT,
            "mask": m01,
            "ones": ones,
        })
        in_maps.append(m)
    return in_maps, None


def kernel(x, rope_cos, rope_sin, w_q, w_k, w_v, w_o):
    from concourse.bass_utils import run_bass_kernel_spmd

    if "nc" not in _compiled:
        _compiled["nc"] = _build_nc()
    nc = _compiled["nc"]

    in_maps, _ = _host_prep(np.asarray(x), np.asarray(rope_cos),
                            np.asarray(rope_sin), np.asarray(w_q),
                            np.asarray(w_k), np.asarray(w_v),
                            np.asarray(w_o))
    res = run_bass_kernel_spmd(nc, in_maps, core_ids=list(range(8)))
    out = np.empty((B, T, D), dtype=np.float32)
    for b in range(B):
        out[b] = (np.asarray(res.results[2 * b]["out"]).astype(np.float32)
                  + np.asarray(res.results[2 * b + 1]["out"]).astype(np.float32))
    return out
